# revision 3
# baseline (speedup 1.0000x reference)
"""Trainium2 Bass kernel for EquivDeepSet RBF grid encoder.

Computes, for each batch b:
    Gram[g, n] = exp(-|grid[g] - X_c[b, n]|^2 / (2 l^2))
    FM[g, c]   = sum_n Gram[g, n] * [1, Y_c[b, n, 0], Y_c[b, n, 1]][c]
    out[b]     = [density, feats / density] reshaped to [3, NY, NX]

Sharding: data-parallel over batch across the 8 cores (B == 8).

Fast path exploits that the grid is a tensor-product mesh:
    Gram[(i, j), n] = Ky[i, n] * Kx[j, n]
with Ky/Kx the 1-D RBF factor matrices [128, 1024].  Per core the whole
computation is then: two [128, 1024] elementwise Gaussians, one
[128n, (3c 128j)] weighted moving matrix, and 8 accumulating matmuls
into a single [128i, 3*128] PSUM tile — no [G, N] Gram materialization.

A general (non-mesh grid) fallback computes the Gram tile-by-tile via a
rank-4 quadratic-form matmul and fuses exp + weighted sum per tile.
"""

import numpy as np
from contextlib import ExitStack

import concourse.bacc as bacc
import concourse.bass as bass
import concourse.tile as tile
from concourse import mybir
from concourse.bass_utils import run_bass_kernel_spmd

F32 = mybir.dt.float32
AF = mybir.ActivationFunctionType
ALU = mybir.AluOpType

B, N, NY, NX = 8, 1024, 128, 128
NCH = N // 128  # n-chunks of 128 context points
G = NY * NX


def _fap(base, dims):
    """AP with the same tensor/partition dim as `base` but custom free dims.

    dims: list of (step, count) pairs in elements of base's layout.
    """
    return bass.AP(
        tensor=base.tensor,
        offset=base.offset,
        ap=[list(base.ap[0])] + [[s, c] for (s, c) in dims],
    )


def _build_separable(inv2l2: float):
    """Per-core program: one batch. Inputs axes[256], xt[128,16], ey[128,24]."""
    nc = bacc.Bacc("TRN2", target_bir_lowering=False, debug=False)
    axes = nc.dram_tensor("axes", [2 * 128], F32, kind="ExternalInput").ap()
    xt = nc.dram_tensor("xt", [128, 2 * NCH], F32, kind="ExternalInput").ap()
    ey = nc.dram_tensor("ey", [128, 3 * NCH], F32, kind="ExternalInput").ap()
    out = nc.dram_tensor("out", [3, NY, NX], F32, kind="ExternalOutput").ap()

    with tile.TileContext(nc) as tc, ExitStack() as ctx:
        singles = ctx.enter_context(tc.tile_pool(name="singles", bufs=1))
        work = ctx.enter_context(tc.tile_pool(name="work", bufs=1))
        psum = ctx.enter_context(tc.tile_pool(name="psum", bufs=1, space="PSUM"))

        # ys/xs broadcast across partitions: axes[0:128]=ys, axes[128:256]=xs
        axes_sb = singles.tile([128, 256], F32)
        nc.sync.dma_start(
            out=axes_sb,
            in_=bass.AP(tensor=axes.tensor, offset=0, ap=[[0, 128], [1, 256]]),
        )
        xt_sb = singles.tile([128, 2 * NCH], F32)
        nc.sync.dma_start(out=xt_sb, in_=xt)
        ey_sb = singles.tile([128, 3 * NCH], F32)
        nc.sync.dma_start(out=ey_sb, in_=ey)

        # Factor matrices, n on partitions: k?t[p, ch, m] = exp(-inv2l2*(axis[m]-X[ch*128+p])^2)
        kyt = work.tile([128, NCH, 128], F32, tag="kyt")
        kxt = work.tile([128, NCH, 128], F32, tag="kxt")
        for (kt, off, xcol) in ((kyt, 0, 1), (kxt, 128, 0)):
            d = work.tile([128, NCH, 128], F32, tag=f"d{xcol}")
            for ch in range(NCH):
                nc.vector.tensor_scalar(
                    out=d[:, ch, :],
                    in0=axes_sb[:, off : off + 128],
                    scalar1=xt_sb[:, 2 * ch + xcol : 2 * ch + xcol + 1],
                    scalar2=None,
                    op0=ALU.subtract,
                )
            df = d.rearrange("p a b -> p (a b)")
            nc.vector.tensor_tensor(out=df, in0=df, in1=df, op=ALU.mult)
            nc.scalar.activation(
                out=kt.rearrange("p a b -> p (a b)"), in_=df, func=AF.Exp, scale=-inv2l2
            )

        # W[p, ch, c, j] = ey[p, 3ch+c] * kxt[p, ch, j]  (one broadcast DVE op)
        w = work.tile([128, NCH, 3, 128], F32, tag="w")
        kx_b = _fap(kxt, [(128, NCH), (0, 3), (1, 128)])
        ey_b = _fap(ey_sb, [(3, NCH), (1, 3), (0, 128)])
        nc.vector.tensor_tensor(out=w, in0=kx_b, in1=ey_b, op=ALU.mult)

        # FM[i, (c, j)] = sum_ch kyt[:, ch, :].T @ w[:, ch, :, :]
        fm = psum.tile([128, 3, 128], F32)
        for ch in range(NCH):
            nc.tensor.matmul(
                fm.rearrange("p c j -> p (c j)"),
                lhsT=kyt[:, ch, :],
                rhs=w[:, ch, :, :].rearrange("p c j -> p (c j)"),
                start=(ch == 0),
                stop=(ch == NCH - 1),
            )

        # normalize: out0 = density, out1/2 = feats / density
        recip = work.tile([128, 128], F32, tag="recip")
        nc.vector.reciprocal(out=recip, in_=fm[:, 0, :])
        osb = work.tile([128, 3, 128], F32, tag="osb")
        nc.scalar.copy(out=osb[:, 0, :], in_=fm[:, 0, :])
        nc.vector.tensor_tensor(out=osb[:, 1, :], in0=fm[:, 1, :], in1=recip, op=ALU.mult)
        nc.vector.tensor_tensor(out=osb[:, 2, :], in0=fm[:, 2, :], in1=recip, op=ALU.mult)

        nc.sync.dma_start(out=out.rearrange("c i j -> i c j"), in_=osb)

    nc.compile()
    return nc


def _run_separable(X_c, Y_c, xs, ys, inv2l2):
    nc = _build_separable(inv2l2)
    axes = np.concatenate([ys, xs]).astype(np.float32)
    # xt[p, 2ch+c] = X_c[b, ch*128+p, c]; ey[p, 3ch+c] = [1, Y0, Y1][c] at n=ch*128+p
    xt = np.ascontiguousarray(
        X_c.reshape(B, NCH, 128, 2).transpose(0, 2, 1, 3).reshape(B, 128, 2 * NCH)
    )
    eyf = np.concatenate([np.ones((B, N, 1), np.float32), Y_c], axis=2)
    ey = np.ascontiguousarray(
        eyf.reshape(B, NCH, 128, 3).transpose(0, 2, 1, 3).reshape(B, 128, 3 * NCH)
    )
    in_maps = [{"axes": axes, "xt": xt[b], "ey": ey[b]} for b in range(B)]
    res = run_bass_kernel_spmd(nc, in_maps, list(range(B)))
    return np.stack([res.results[b]["out"] for b in range(B)]).astype(np.float32)


def _build_general(inv2l2: float):
    """Fallback for a non-mesh grid: quadratic-form Gram, fused per g-tile.

    Inputs per core: xa[4, 1024] (X augmented), ga[4, G] (grid augmented),
    eyt[128, 3*NCH].  S[n, g] = sum_k xa[k, n] * ga[k, g] = -d2/(2 l^2).
    """
    nc = bacc.Bacc("TRN2", target_bir_lowering=False, debug=False)
    xa = nc.dram_tensor("xa", [4, N], F32, kind="ExternalInput").ap()
    ga = nc.dram_tensor("ga", [4, G], F32, kind="ExternalInput").ap()
    ey = nc.dram_tensor("ey", [128, 3 * NCH], F32, kind="ExternalInput").ap()
    out = nc.dram_tensor("out", [3, G], F32, kind="ExternalOutput").ap()

    GT = 512  # g-tile width
    with tile.TileContext(nc) as tc, ExitStack() as ctx:
        singles = ctx.enter_context(tc.tile_pool(name="singles", bufs=1))
        gpool = ctx.enter_context(tc.tile_pool(name="gpool", bufs=2))
        gram_p = ctx.enter_context(tc.tile_pool(name="gram_p", bufs=2))
        spsum = ctx.enter_context(tc.tile_pool(name="spsum", bufs=4, space="PSUM"))
        fpsum = ctx.enter_context(tc.tile_pool(name="fpsum", bufs=2, space="PSUM"))
        opool = ctx.enter_context(tc.tile_pool(name="opool", bufs=2))

        xa_sb = singles.tile([4, N], F32)
        nc.sync.dma_start(out=xa_sb, in_=xa)
        ey_sb = singles.tile([128, 3 * NCH], F32)
        nc.sync.dma_start(out=ey_sb, in_=ey)

        for gt in range(G // GT):
            ga_sb = gpool.tile([4, GT], F32, tag="ga")
            nc.sync.dma_start(out=ga_sb, in_=ga[:, gt * GT : (gt + 1) * GT])
            fm = fpsum.tile([128, GT // 128, 3], F32, tag="fm")
            for ch in range(NCH):
                s = spsum.tile([128, GT], F32, tag="s")
                nc.tensor.matmul(
                    s, lhsT=xa_sb[:, ch * 128 : (ch + 1) * 128], rhs=ga_sb,
                    start=True, stop=True,
                )
                gram = gram_p.tile([128, GT], F32, tag="gram")
                nc.scalar.activation(out=gram, in_=s, func=AF.Exp, scale=1.0)
                for sub in range(GT // 128):
                    nc.tensor.matmul(
                        fm[:, sub, :],
                        lhsT=gram[:, sub * 128 : (sub + 1) * 128],
                        rhs=ey_sb[:, 3 * ch : 3 * ch + 3],
                        start=(ch == 0),
                        stop=(ch == NCH - 1),
                    )
            # fm[gp, sub, c] for g = gt*GT + sub*128 + gp
            recip = opool.tile([128, GT // 128], F32, tag="recip")
            nc.vector.reciprocal(out=recip, in_=fm[:, :, 0])
            osb = opool.tile([128, 3, GT // 128], F32, tag="osb")
            nc.scalar.copy(out=osb[:, 0, :], in_=fm[:, :, 0])
            for c in (1, 2):
                nc.vector.tensor_tensor(
                    out=osb[:, c, :], in0=fm[:, :, c], in1=recip, op=ALU.mult
                )
            # out[c, g]: g = gt*GT + sub*128 + gp -> per channel dram dims [gp, sub]
            for c in range(3):
                dview = bass.AP(
                    tensor=out.tensor,
                    offset=c * G + gt * GT,
                    ap=[[1, 128], [128, GT // 128]],
                )
                nc.sync.dma_start(out=dview, in_=osb[:, c, :])

    nc.compile()
    return nc


def _run_general(X_c, Y_c, grid, inv2l2):
    nc = _build_general(inv2l2)
    Xs = (X_c * X_c).sum(-1)  # [B, N]
    xa = np.stack(
        [X_c[..., 0], X_c[..., 1], -inv2l2 * Xs, np.ones((B, N), np.float32)], axis=1
    ).astype(np.float32)  # [B, 4, N]
    gs = (grid * grid).sum(-1)  # [G]
    ga = np.stack(
        [
            2.0 * inv2l2 * grid[:, 0],
            2.0 * inv2l2 * grid[:, 1],
            np.ones(G, np.float32),
            -inv2l2 * gs,
        ],
        axis=0,
    ).astype(np.float32)  # [4, G]
    eyf = np.concatenate([np.ones((B, N, 1), np.float32), Y_c], axis=2)
    ey = np.ascontiguousarray(
        eyf.reshape(B, NCH, 128, 3).transpose(0, 2, 1, 3).reshape(B, 128, 3 * NCH)
    )
    in_maps = [{"xa": xa[b], "ga": ga, "ey": ey[b]} for b in range(B)]
    res = run_bass_kernel_spmd(nc, in_maps, list(range(B)))
    fm = np.stack([res.results[b]["out"] for b in range(B)])  # [B, 3, G]
    return fm.reshape(B, 3, NY, NX).astype(np.float32)


def kernel(X_c, Y_c, grid, log_l_scale):
    X_c = np.asarray(X_c, np.float32)
    Y_c = np.asarray(Y_c, np.float32)
    grid = np.asarray(grid, np.float32)
    l = float(np.exp(np.clip(np.asarray(log_l_scale)[0], -5.0, 5.0)))
    inv2l2 = 1.0 / (2.0 * l * l)

    grid_r = grid.reshape(NY, NX, 2)
    xs = grid_r[0, :, 0].copy()
    ys = grid_r[:, 0, 1].copy()
    separable = np.array_equal(grid_r[..., 0], np.broadcast_to(xs[None, :], (NY, NX))) and (
        np.array_equal(grid_r[..., 1], np.broadcast_to(ys[:, None], (NY, NX)))
    )
    if separable:
        return _run_separable(X_c, Y_c, xs, ys, inv2l2)
    return _run_general(X_c, Y_c, grid, inv2l2)


if __name__ == "__main__":
    rng = np.random.default_rng(0)
    ins = {
        "X_c": rng.uniform(0, 10, (B, N, 2)).astype(np.float32),
        "Y_c": rng.normal(size=(B, N, 2)).astype(np.float32),
        "grid": np.stack(
            np.meshgrid(
                np.linspace(0, 10, NY, dtype=np.float32),
                np.linspace(0, 10, NX, dtype=np.float32),
                indexing="ij",
            )[::-1],
            axis=-1,
        ).reshape(-1, 2),
        "log_l_scale": np.zeros(1, np.float32),
    }
    o = kernel(**ins)
    print("out", o.shape, o.dtype)


# revision 26
# speedup vs baseline: 17758.1122x; 17758.1122x over previous
"""Trainium2 Bass kernel for EquivDeepSet RBF grid encoder.

Computes, for each batch b:
    Gram[g, n] = exp(-|grid[g] - X_c[b, n]|^2 / (2 l^2))
    FM[g, c]   = sum_n Gram[g, n] * [1, Y_c[b, n, 0], Y_c[b, n, 1]][c]
    out[b]     = [density, feats / density] reshaped to [3, NY, NX]

Sharding: data-parallel over batch across the 8 cores (B == 8).

Fast path exploits that the grid is a tensor-product mesh:
    Gram[(i, j), n] = Ky[i, n] * Kx[j, n]
with Ky/Kx the 1-D RBF factor matrices [128, 1024].  Per core the whole
computation is then: two [128, 1024] elementwise Gaussians, one
[128n, (3c 128j)] weighted moving matrix, and 8 accumulating matmuls
into a single [128i, 3*128] PSUM tile — no [G, N] Gram materialization.

A general (non-mesh grid) fallback computes the Gram tile-by-tile via a
rank-4 quadratic-form matmul and fuses exp + weighted sum per tile.
"""

import numpy as np
from contextlib import ExitStack

import concourse.bacc as bacc
import concourse.bass as bass
import concourse.tile as tile
from concourse import mybir
from concourse.bass_utils import run_bass_kernel_spmd

F32 = mybir.dt.float32
AF = mybir.ActivationFunctionType
ALU = mybir.AluOpType

B, N, NY, NX = 8, 1024, 128, 128
NCH = N // 128  # n-chunks of 128 context points
G = NY * NX


def _fap(base, dims, extra_offset=0):
    """AP with the same tensor/partition dim as `base` but custom free dims.

    dims: list of (step, count) pairs in elements of base's layout.
    """
    return bass.AP(
        tensor=base.tensor,
        offset=base.offset + extra_offset,
        ap=[list(base.ap[0])] + [[s, c] for (s, c) in dims],
    )


def _build_separable(inv2l2: float, step_y: float, step_x: float,
                     sqy_eng=("act", "act", "act", "act", "dve", "act", "dve", "dve"),
                     sqx_eng=("pool", "dve", "pool", "dve", "pool", "dve", "pool", "pool"),
                     exp_pairs=True, loop_k=None):
    """Per-core program: one batch.

    Inputs: blob[128,40] f32: cols 0:8 = by (ys0 - Xy per chunk),
    8:16 = bx (xs0 - Xx per chunk), 16:40 = expanded-Y fp32.

    The grid axes are affine (ys[i] = ys0 + i*step_y), so a Pool iota ramp
    replaces any axes input: d = step*iota + bias, squared, then
    exp(-inv2l2 * d^2) in bf16.  w[c,j] = ey[c]*kx[j] via bf16
    tensor_scalar; fm[i,(c,j)] accumulates over the 8 n-chunks in PSUM.
    """
    nc = bacc.Bacc("TRN2", target_bir_lowering=False, debug=False)
    BF16 = mybir.dt.bfloat16
    blob = nc.dram_tensor("blob", [128, 40], F32, kind="ExternalInput").ap()
    out = nc.dram_tensor("out", [3, NY, NX], F32, kind="ExternalOutput").ap()

    with tile.TileContext(nc) as tc, ExitStack() as ctx:
        singles = ctx.enter_context(tc.tile_pool(name="singles", bufs=1))
        work = ctx.enter_context(tc.tile_pool(name="work", bufs=3))
        opool = ctx.enter_context(tc.tile_pool(name="opool", bufs=1))
        psum = ctx.enter_context(tc.tile_pool(name="psum", bufs=1, space="PSUM"))

        blob_sb = singles.tile([128, 40], F32)
        nc.sync.dma_start(out=blob_sb, in_=blob)
        ey_sb = blob_sb[:, 16:40]  # [128, 24] fp32

        # Early dummy activations force the (Square|Exp) table load to start
        # before any data dependency, overlapping the blob DMA.
        scr = singles.tile([1, 8], F32)
        nc.vector.memset(scr, 0.0)
        scr2 = singles.tile([1, 8], F32)
        nc.scalar.activation(out=scr2, in_=scr, func=AF.Square, scale=1.0)

        iota = singles.tile([128, 128], F32)
        nc.gpsimd.iota(
            iota, [[1, 128]], channel_multiplier=0,
            allow_small_or_imprecise_dtypes=True,  # 0..127 exact in fp32
        )

        def emit_square(eng_name, d_out, step, bias_ap, tag):
            """d_out = (step*iota + bias)^2 on the chosen engine."""
            if eng_name == "act":
                nc.scalar.activation(
                    out=d_out, in_=iota, func=AF.Square, bias=bias_ap, scale=step
                )
            else:
                eng = nc.gpsimd if eng_name == "pool" else nc.vector
                d = work.tile([128, 128], F32, tag=tag)
                eng.tensor_scalar(
                    out=d, in0=iota, scalar1=step, scalar2=bias_ap,
                    op0=ALU.mult, op1=ALU.add,
                )
                eng.tensor_tensor(out=d_out, in0=d, in1=d, op=ALU.mult)

        loop_ctx = tc.For_i(0, loop_k, 1) if loop_k else None
        if loop_ctx is None:
            fm_d = psum.tile([128, 128], F32, tag="fm_d")
            fm_f = psum.tile([128, 2, 128], F32, tag="fm_f")
            if exp_pairs == "mixed":
                groups = [(0, 1), (2, 3), (4, 5), (6,), (7,)]
            elif exp_pairs:
                groups = [(0, 1), (2, 3), (4, 5), (6, 7)]
            else:
                groups = [(ch,) for ch in range(NCH)]
            for grp in groups:
                per = len(grp)
                dsq = work.tile([128, per, 2, 128], F32, tag=f"dsq{per}")
                kk = work.tile([128, per, 2, 128], BF16, tag=f"kk{per}")
                for half, ch in enumerate(grp):
                    emit_square(sqy_eng[ch], dsq[:, half, 0, :], step_y,
                                blob_sb[:, ch : ch + 1], f"dy{half}")
                    emit_square(sqx_eng[ch], dsq[:, half, 1, :], step_x,
                                blob_sb[:, 8 + ch : 8 + ch + 1], f"dx{half}")
                nc.scalar.activation(
                    out=kk.rearrange("p a b c -> p (a b c)"),
                    in_=dsq.rearrange("p a b c -> p (a b c)"),
                    func=AF.Exp,
                    scale=-inv2l2,
                )
                for half, ch in enumerate(grp):
                    # density channel: ey[:, 3ch] == 1, so rhs is Kx itself
                    nc.tensor.matmul(
                        fm_d,
                        lhsT=kk[:, half, 0, :],
                        rhs=kk[:, half, 1, :],
                        start=(ch == 0),
                        stop=(ch == NCH - 1),
                    )
                    w = work.tile([128, 2, 128], BF16, tag="w")
                    for c in (1, 2):
                        nc.vector.tensor_scalar(
                            out=w[:, c - 1, :],
                            in0=kk[:, half, 1, :],
                            scalar1=ey_sb[:, 3 * ch + c : 3 * ch + c + 1],
                            scalar2=None,
                            op0=ALU.mult,
                        )
                    nc.tensor.matmul(
                        fm_f.rearrange("p c j -> p (c j)"),
                        lhsT=kk[:, half, 0, :],
                        rhs=w.rearrange("p c j -> p (c j)"),
                        start=(ch == 0),
                        stop=(ch == NCH - 1),
                    )

            # normalize: out0 = density, out1/2 = feats * (1/density)
            osb = opool.tile([128, 3, 128], F32, tag="osb")
            nc.scalar.copy(out=osb[:, 0, :], in_=fm_d)
            recip = opool.tile([128, 128], F32, tag="recip")
            nc.vector.reciprocal(out=recip, in_=fm_d)
            nc.vector.scalar_tensor_tensor(
                out=osb[:, 1:3, :],
                in0=fm_f,
                scalar=1.0,
                in1=_fap(recip, [(0, 2), (1, 128)]),
                op0=ALU.mult,
                op1=ALU.mult,
            )
            nc.sync.dma_start(
                out=bass.AP(tensor=out.tensor, offset=0, ap=[[NX, NY], [G, 3], [1, NX]]),
                in_=osb,
            )
        else:
            with loop_ctx:
                fm_d = psum.tile([128, 128], F32, tag="fm_d")
                fm_f = psum.tile([128, 2, 128], F32, tag="fm_f")
                if exp_pairs == "mixed":
                    groups = [(0, 1), (2, 3), (4, 5), (6,), (7,)]
                elif exp_pairs:
                    groups = [(0, 1), (2, 3), (4, 5), (6, 7)]
                else:
                    groups = [(ch,) for ch in range(NCH)]
                for grp in groups:
                    per = len(grp)
                    dsq = work.tile([128, per, 2, 128], F32, tag=f"dsq{per}")
                    kk = work.tile([128, per, 2, 128], BF16, tag=f"kk{per}")
                    for half, ch in enumerate(grp):
                        emit_square(sqy_eng[ch], dsq[:, half, 0, :], step_y,
                                    blob_sb[:, ch : ch + 1], f"dy{half}")
                        emit_square(sqx_eng[ch], dsq[:, half, 1, :], step_x,
                                    blob_sb[:, 8 + ch : 8 + ch + 1], f"dx{half}")
                    nc.scalar.activation(
                        out=kk.rearrange("p a b c -> p (a b c)"),
                        in_=dsq.rearrange("p a b c -> p (a b c)"),
                        func=AF.Exp,
                        scale=-inv2l2,
                    )
                    for half, ch in enumerate(grp):
                        # density channel: ey[:, 3ch] == 1, so rhs is Kx itself
                        nc.tensor.matmul(
                            fm_d,
                            lhsT=kk[:, half, 0, :],
                            rhs=kk[:, half, 1, :],
                            start=(ch == 0),
                            stop=(ch == NCH - 1),
                        )
                        w = work.tile([128, 2, 128], BF16, tag="w")
                        for c in (1, 2):
                            nc.vector.tensor_scalar(
                                out=w[:, c - 1, :],
                                in0=kk[:, half, 1, :],
                                scalar1=ey_sb[:, 3 * ch + c : 3 * ch + c + 1],
                                scalar2=None,
                                op0=ALU.mult,
                            )
                        nc.tensor.matmul(
                            fm_f.rearrange("p c j -> p (c j)"),
                            lhsT=kk[:, half, 0, :],
                            rhs=w.rearrange("p c j -> p (c j)"),
                            start=(ch == 0),
                            stop=(ch == NCH - 1),
                        )

                # normalize: out0 = density, out1/2 = feats * (1/density)
                osb = opool.tile([128, 3, 128], F32, tag="osb")
                nc.scalar.copy(out=osb[:, 0, :], in_=fm_d)
                recip = opool.tile([128, 128], F32, tag="recip")
                nc.vector.reciprocal(out=recip, in_=fm_d)
                nc.vector.scalar_tensor_tensor(
                    out=osb[:, 1:3, :],
                    in0=fm_f,
                    scalar=1.0,
                    in1=_fap(recip, [(0, 2), (1, 128)]),
                    op0=ALU.mult,
                    op1=ALU.mult,
                )
                nc.sync.dma_start(
                    out=bass.AP(tensor=out.tensor, offset=0, ap=[[NX, NY], [G, 3], [1, NX]]),
                    in_=osb,
                )

    nc.compile()
    return nc


def axes_affine(v):
    """(v0, step) if v is an affine fp32 ramp v0 + i*step (to ~1 ulp), else None."""
    v = np.asarray(v, np.float32)
    n = v.shape[0]
    step = np.float32((float(v[-1]) - float(v[0])) / (n - 1))
    gen = (np.float32(v[0]) + np.arange(n, dtype=np.float32) * step).astype(np.float32)
    span = max(abs(float(v[-1] - v[0])), 1e-30)
    if np.abs(gen - v).max() <= 1e-6 * span:
        return float(v[0]), float(step)
    return None


def make_separable_inputs(X_c, Y_c, ys0, xs0):
    # by[p, ch] = ys0 - Xy[ch*128+p];  bx[p, ch] = xs0 - Xx[ch*128+p]
    xt = X_c.reshape(B, NCH, 128, 2).transpose(0, 2, 1, 3)  # [B,128,NCH,2]
    by = np.float32(ys0) - xt[..., 1]
    bx = np.float32(xs0) - xt[..., 0]
    eyf = np.concatenate([np.ones((B, N, 1), np.float32), Y_c], axis=2)
    ey = np.ascontiguousarray(
        eyf.reshape(B, NCH, 128, 3).transpose(0, 2, 1, 3).reshape(B, 128, 3 * NCH)
    )
    blob = np.concatenate([by, bx, ey], axis=2).astype(np.float32)  # [B,128,40]
    return [{"blob": blob[b]} for b in range(B)]


_program_cache = {}


def _cached_separable(inv2l2, step_y, step_x):
    key = ("sep", inv2l2, step_y, step_x)
    if key not in _program_cache:
        _program_cache[key] = _build_separable(inv2l2, step_y, step_x)
    return _program_cache[key]


def _run_separable(X_c, Y_c, affy, affx, inv2l2):
    (ys0, step_y), (xs0, step_x) = affy, affx
    nc = _cached_separable(inv2l2, step_y, step_x)
    in_maps = make_separable_inputs(X_c, Y_c, ys0, xs0)
    res = run_bass_kernel_spmd(nc, in_maps, list(range(B)))
    return np.stack([res.results[b]["out"] for b in range(B)]).astype(np.float32)


def _build_general(inv2l2: float):
    """Fallback for a non-mesh grid: quadratic-form Gram, fused per g-tile.

    Inputs per core: xa[4, 1024] (X augmented), ga[4, G] (grid augmented),
    eyt[128, 3*NCH].  S[n, g] = sum_k xa[k, n] * ga[k, g] = -d2/(2 l^2).
    """
    nc = bacc.Bacc("TRN2", target_bir_lowering=False, debug=False)
    xa = nc.dram_tensor("xa", [4, N], F32, kind="ExternalInput").ap()
    ga = nc.dram_tensor("ga", [4, G], F32, kind="ExternalInput").ap()
    ey = nc.dram_tensor("ey", [128, 3 * NCH], F32, kind="ExternalInput").ap()
    out = nc.dram_tensor("out", [3, G], F32, kind="ExternalOutput").ap()

    GT = 512  # g-tile width
    with tile.TileContext(nc) as tc, ExitStack() as ctx:
        singles = ctx.enter_context(tc.tile_pool(name="singles", bufs=1))
        gpool = ctx.enter_context(tc.tile_pool(name="gpool", bufs=2))
        gram_p = ctx.enter_context(tc.tile_pool(name="gram_p", bufs=2))
        spsum = ctx.enter_context(tc.tile_pool(name="spsum", bufs=2, space="PSUM"))
        fpsum = ctx.enter_context(tc.tile_pool(name="fpsum", bufs=1, space="PSUM"))
        opool = ctx.enter_context(tc.tile_pool(name="opool", bufs=2))

        xa_sb = singles.tile([4, N], F32)
        nc.sync.dma_start(out=xa_sb, in_=xa)
        ey_sb = singles.tile([128, 3 * NCH], F32)
        nc.sync.dma_start(out=ey_sb, in_=ey)

        for gt in range(G // GT):
            ga_sb = gpool.tile([4, GT], F32, tag="ga")
            nc.sync.dma_start(out=ga_sb, in_=ga[:, gt * GT : (gt + 1) * GT])
            # one PSUM tile per 128-g subtile: interleaved matmul accumulation
            # groups must not share a PSUM zero region
            fms = [
                fpsum.tile([128, 3], F32, tag=f"fm{sub}", name=f"fm{sub}_{gt}")
                for sub in range(GT // 128)
            ]
            for ch in range(NCH):
                s = spsum.tile([128, GT], F32, tag="s")
                nc.tensor.matmul(
                    s, lhsT=xa_sb[:, ch * 128 : (ch + 1) * 128], rhs=ga_sb,
                    start=True, stop=True,
                )
                gram = gram_p.tile([128, GT], F32, tag="gram")
                nc.scalar.activation(out=gram, in_=s, func=AF.Exp, scale=1.0)
                for sub in range(GT // 128):
                    nc.tensor.matmul(
                        fms[sub],
                        lhsT=gram[:, sub * 128 : (sub + 1) * 128],
                        rhs=ey_sb[:, 3 * ch : 3 * ch + 3],
                        start=(ch == 0),
                        stop=(ch == NCH - 1),
                    )
            # fms[sub][gp, c] for g = gt*GT + sub*128 + gp
            recip = opool.tile([128, GT // 128], F32, tag="recip")
            osb = opool.tile([128, 3, GT // 128], F32, tag="osb")
            for sub in range(GT // 128):
                nc.vector.reciprocal(
                    out=recip[:, sub : sub + 1], in_=fms[sub][:, 0:1]
                )
                nc.scalar.copy(out=osb[:, 0, sub : sub + 1], in_=fms[sub][:, 0:1])
                for c in (1, 2):
                    nc.vector.tensor_tensor(
                        out=osb[:, c, sub : sub + 1],
                        in0=fms[sub][:, c : c + 1],
                        in1=recip[:, sub : sub + 1],
                        op=ALU.mult,
                    )
            # out[c, g]: g = gt*GT + sub*128 + gp -> per channel dram dims [gp, sub]
            for c in range(3):
                dview = bass.AP(
                    tensor=out.tensor,
                    offset=c * G + gt * GT,
                    ap=[[1, 128], [128, GT // 128]],
                )
                nc.sync.dma_start(out=dview, in_=osb[:, c, :])

    nc.compile()
    return nc


def _run_general(X_c, Y_c, grid, inv2l2):
    nc = _build_general(inv2l2)
    Xs = (X_c * X_c).sum(-1)  # [B, N]
    xa = np.stack(
        [X_c[..., 0], X_c[..., 1], -inv2l2 * Xs, np.ones((B, N), np.float32)], axis=1
    ).astype(np.float32)  # [B, 4, N]
    gs = (grid * grid).sum(-1)  # [G]
    ga = np.stack(
        [
            2.0 * inv2l2 * grid[:, 0],
            2.0 * inv2l2 * grid[:, 1],
            np.ones(G, np.float32),
            -inv2l2 * gs,
        ],
        axis=0,
    ).astype(np.float32)  # [4, G]
    eyf = np.concatenate([np.ones((B, N, 1), np.float32), Y_c], axis=2)
    ey = np.ascontiguousarray(
        eyf.reshape(B, NCH, 128, 3).transpose(0, 2, 1, 3).reshape(B, 128, 3 * NCH)
    )
    in_maps = [{"xa": xa[b], "ga": ga, "ey": ey[b]} for b in range(B)]
    res = run_bass_kernel_spmd(nc, in_maps, list(range(B)))
    fm = np.stack([res.results[b]["out"] for b in range(B)])  # [B, 3, G]
    return fm.reshape(B, 3, NY, NX).astype(np.float32)


def kernel(X_c, Y_c, grid, log_l_scale):
    X_c = np.asarray(X_c, np.float32)
    Y_c = np.asarray(Y_c, np.float32)
    grid = np.asarray(grid, np.float32)
    l = float(np.exp(np.clip(np.asarray(log_l_scale)[0], -5.0, 5.0)))
    inv2l2 = 1.0 / (2.0 * l * l)

    grid_r = grid.reshape(NY, NX, 2)
    xs = grid_r[0, :, 0].copy()
    ys = grid_r[:, 0, 1].copy()
    separable = np.array_equal(grid_r[..., 0], np.broadcast_to(xs[None, :], (NY, NX))) and (
        np.array_equal(grid_r[..., 1], np.broadcast_to(ys[:, None], (NY, NX)))
    )
    affy, affx = axes_affine(ys), axes_affine(xs)
    if separable and affy is not None and affx is not None:
        return _run_separable(X_c, Y_c, affy, affx, inv2l2)
    return _run_general(X_c, Y_c, grid, inv2l2)


if __name__ == "__main__":
    rng = np.random.default_rng(0)
    ins = {
        "X_c": rng.uniform(0, 10, (B, N, 2)).astype(np.float32),
        "Y_c": rng.normal(size=(B, N, 2)).astype(np.float32),
        "grid": np.stack(
            np.meshgrid(
                np.linspace(0, 10, NY, dtype=np.float32),
                np.linspace(0, 10, NX, dtype=np.float32),
                indexing="ij",
            )[::-1],
            axis=-1,
        ).reshape(-1, 2),
        "log_l_scale": np.zeros(1, np.float32),
    }
    o = kernel(**ins)
    print("out", o.shape, o.dtype)



# revision 28
# speedup vs baseline: 18848.9295x; 1.0614x over previous
"""Trainium2 Bass kernel for EquivDeepSet RBF grid encoder.

Computes, for each batch b:
    Gram[g, n] = exp(-|grid[g] - X_c[b, n]|^2 / (2 l^2))
    FM[g, c]   = sum_n Gram[g, n] * [1, Y_c[b, n, 0], Y_c[b, n, 1]][c]
    out[b]     = [density, feats / density] reshaped to [3, NY, NX]

Sharding: data-parallel over batch across the 8 cores (B == 8).

Fast path exploits that the grid is a tensor-product mesh:
    Gram[(i, j), n] = Ky[i, n] * Kx[j, n]
with Ky/Kx the 1-D RBF factor matrices [128, 1024].  Per core the whole
computation is then: two [128, 1024] elementwise Gaussians, one
[128n, (3c 128j)] weighted moving matrix, and 8 accumulating matmuls
into a single [128i, 3*128] PSUM tile — no [G, N] Gram materialization.

A general (non-mesh grid) fallback computes the Gram tile-by-tile via a
rank-4 quadratic-form matmul and fuses exp + weighted sum per tile.
"""

import numpy as np
from contextlib import ExitStack

import concourse.bacc as bacc
import concourse.bass as bass
import concourse.tile as tile
from concourse import mybir
from concourse.bass_utils import run_bass_kernel_spmd

F32 = mybir.dt.float32
AF = mybir.ActivationFunctionType
ALU = mybir.AluOpType

B, N, NY, NX = 8, 1024, 128, 128
NCH = N // 128  # n-chunks of 128 context points
G = NY * NX


def _fap(base, dims, extra_offset=0):
    """AP with the same tensor/partition dim as `base` but custom free dims.

    dims: list of (step, count) pairs in elements of base's layout.
    """
    return bass.AP(
        tensor=base.tensor,
        offset=base.offset + extra_offset,
        ap=[list(base.ap[0])] + [[s, c] for (s, c) in dims],
    )


def _build_separable(inv2l2: float, step_y: float, step_x: float,
                     sqy_eng=("act", "act", "act", "act", "dve", "act", "dve", "dve"),
                     sqx_eng=("pool", "dve", "pool", "dve", "pool", "dve", "pool", "pool"),
                     exp_pairs=True, loop_k=None):
    """Per-core program: one batch.

    Inputs: blob[128,40] f32: cols 0:8 = by (ys0 - Xy per chunk),
    8:16 = bx (xs0 - Xx per chunk), 16:40 = expanded-Y fp32.

    The grid axes are affine (ys[i] = ys0 + i*step_y), so a Pool iota ramp
    replaces any axes input: d = step*iota + bias, squared, then
    exp(-inv2l2 * d^2) in bf16.  w[c,j] = ey[c]*kx[j] via bf16
    tensor_scalar; fm[i,(c,j)] accumulates over the 8 n-chunks in PSUM.
    """
    nc = bacc.Bacc("TRN2", target_bir_lowering=False, debug=False)
    BF16 = mybir.dt.bfloat16
    blob = nc.dram_tensor("blob", [128, 40], F32, kind="ExternalInput").ap()
    out = nc.dram_tensor("out", [3, NY, NX], F32, kind="ExternalOutput").ap()

    with tile.TileContext(nc) as tc, ExitStack() as ctx:
        singles = ctx.enter_context(tc.tile_pool(name="singles", bufs=1))
        work = ctx.enter_context(tc.tile_pool(name="work", bufs=3))
        opool = ctx.enter_context(tc.tile_pool(name="opool", bufs=1))
        psum = ctx.enter_context(tc.tile_pool(name="psum", bufs=1, space="PSUM"))

        blob_sb = singles.tile([128, 40], F32)
        nc.sync.dma_start(out=blob_sb, in_=blob)
        ey_sb = blob_sb[:, 16:40]  # [128, 24] fp32

        # Early dummy activations force the (Square|Exp) table load to start
        # before any data dependency, overlapping the blob DMA.
        scr = singles.tile([1, 8], F32)
        nc.vector.memset(scr, 0.0)
        scr2 = singles.tile([1, 8], F32)
        nc.scalar.activation(out=scr2, in_=scr, func=AF.Square, scale=1.0)

        iota = singles.tile([128, 128], F32)
        nc.gpsimd.iota(
            iota, [[1, 128]], channel_multiplier=0,
            allow_small_or_imprecise_dtypes=True,  # 0..127 exact in fp32
        )

        def emit_square(eng_name, d_out, step, bias_ap, tag):
            """d_out = (step*iota + bias)^2 on the chosen engine."""
            if eng_name == "act":
                nc.scalar.activation(
                    out=d_out, in_=iota, func=AF.Square, bias=bias_ap, scale=step
                )
            else:
                eng = nc.gpsimd if eng_name == "pool" else nc.vector
                d = work.tile([128, 128], F32, tag=tag)
                eng.tensor_scalar(
                    out=d, in0=iota, scalar1=step, scalar2=bias_ap,
                    op0=ALU.mult, op1=ALU.add,
                )
                eng.tensor_tensor(out=d_out, in0=d, in1=d, op=ALU.mult)

        loop_ctx = tc.For_i(0, loop_k, 1) if loop_k else None
        if loop_ctx is None:
            fm_d = psum.tile([128, 128], F32, tag="fm_d")
            fm_f = psum.tile([128, 2, 128], F32, tag="fm_f")
            if exp_pairs == "mixed":
                groups = [(0, 1), (2, 3), (4, 5), (6,), (7,)]
            elif exp_pairs:
                groups = [(0, 1), (2, 3), (4, 5), (6, 7)]
            else:
                groups = [(ch,) for ch in range(NCH)]
            for grp in groups:
                per = len(grp)
                dsq = work.tile([128, per, 2, 128], F32, tag=f"dsq{per}")
                kk = work.tile([128, per, 2, 128], BF16, tag=f"kk{per}")
                for half, ch in enumerate(grp):
                    emit_square(sqy_eng[ch], dsq[:, half, 0, :], step_y,
                                blob_sb[:, ch : ch + 1], f"dy{half}")
                    emit_square(sqx_eng[ch], dsq[:, half, 1, :], step_x,
                                blob_sb[:, 8 + ch : 8 + ch + 1], f"dx{half}")
                nc.scalar.activation(
                    out=kk.rearrange("p a b c -> p (a b c)"),
                    in_=dsq.rearrange("p a b c -> p (a b c)"),
                    func=AF.Exp,
                    scale=-inv2l2,
                )
                for half, ch in enumerate(grp):
                    # density channel: ey[:, 3ch] == 1, so rhs is Kx itself
                    nc.tensor.matmul(
                        fm_d,
                        lhsT=kk[:, half, 0, :],
                        rhs=kk[:, half, 1, :],
                        start=(ch == 0),
                        stop=(ch == NCH - 1),
                    )
                    w = work.tile([128, 2, 128], BF16, tag="w")
                    for c in (1, 2):
                        nc.vector.tensor_scalar(
                            out=w[:, c - 1, :],
                            in0=kk[:, half, 1, :],
                            scalar1=ey_sb[:, 3 * ch + c : 3 * ch + c + 1],
                            scalar2=None,
                            op0=ALU.mult,
                        )
                    nc.tensor.matmul(
                        fm_f.rearrange("p c j -> p (c j)"),
                        lhsT=kk[:, half, 0, :],
                        rhs=w.rearrange("p c j -> p (c j)"),
                        start=(ch == 0),
                        stop=(ch == NCH - 1),
                    )

            # normalize: out0 = density, out1/2 = feats * (1/density)
            osb = opool.tile([128, 3, 128], F32, tag="osb")
            nc.scalar.copy(out=osb[:, 0, :], in_=fm_d)
            recip = opool.tile([128, 128], F32, tag="recip")
            nc.vector.reciprocal(out=recip, in_=fm_d)
            nc.vector.scalar_tensor_tensor(
                out=osb[:, 1:3, :],
                in0=fm_f,
                scalar=1.0,
                in1=_fap(recip, [(0, 2), (1, 128)]),
                op0=ALU.mult,
                op1=ALU.mult,
            )
            nc.sync.dma_start(
                out=bass.AP(tensor=out.tensor, offset=0, ap=[[NX, NY], [G, 3], [1, NX]]),
                in_=osb,
            )
        else:
            with loop_ctx:
                fm_d = psum.tile([128, 128], F32, tag="fm_d")
                fm_f = psum.tile([128, 2, 128], F32, tag="fm_f")
                if exp_pairs == "mixed":
                    groups = [(0, 1), (2, 3), (4, 5), (6,), (7,)]
                elif exp_pairs:
                    groups = [(0, 1), (2, 3), (4, 5), (6, 7)]
                else:
                    groups = [(ch,) for ch in range(NCH)]
                for grp in groups:
                    per = len(grp)
                    dsq = work.tile([128, per, 2, 128], F32, tag=f"dsq{per}")
                    kk = work.tile([128, per, 2, 128], BF16, tag=f"kk{per}")
                    for half, ch in enumerate(grp):
                        emit_square(sqy_eng[ch], dsq[:, half, 0, :], step_y,
                                    blob_sb[:, ch : ch + 1], f"dy{half}")
                        emit_square(sqx_eng[ch], dsq[:, half, 1, :], step_x,
                                    blob_sb[:, 8 + ch : 8 + ch + 1], f"dx{half}")
                    nc.scalar.activation(
                        out=kk.rearrange("p a b c -> p (a b c)"),
                        in_=dsq.rearrange("p a b c -> p (a b c)"),
                        func=AF.Exp,
                        scale=-inv2l2,
                    )
                    for half, ch in enumerate(grp):
                        # density channel: ey[:, 3ch] == 1, so rhs is Kx itself
                        nc.tensor.matmul(
                            fm_d,
                            lhsT=kk[:, half, 0, :],
                            rhs=kk[:, half, 1, :],
                            start=(ch == 0),
                            stop=(ch == NCH - 1),
                        )
                        w = work.tile([128, 2, 128], BF16, tag="w")
                        for c in (1, 2):
                            nc.vector.tensor_scalar(
                                out=w[:, c - 1, :],
                                in0=kk[:, half, 1, :],
                                scalar1=ey_sb[:, 3 * ch + c : 3 * ch + c + 1],
                                scalar2=None,
                                op0=ALU.mult,
                            )
                        nc.tensor.matmul(
                            fm_f.rearrange("p c j -> p (c j)"),
                            lhsT=kk[:, half, 0, :],
                            rhs=w.rearrange("p c j -> p (c j)"),
                            start=(ch == 0),
                            stop=(ch == NCH - 1),
                        )

                # normalize: out0 = density, out1/2 = feats * (1/density)
                osb = opool.tile([128, 3, 128], F32, tag="osb")
                nc.scalar.copy(out=osb[:, 0, :], in_=fm_d)
                recip = opool.tile([128, 128], F32, tag="recip")
                nc.vector.reciprocal(out=recip, in_=fm_d)
                nc.vector.scalar_tensor_tensor(
                    out=osb[:, 1:3, :],
                    in0=fm_f,
                    scalar=1.0,
                    in1=_fap(recip, [(0, 2), (1, 128)]),
                    op0=ALU.mult,
                    op1=ALU.mult,
                )
                nc.sync.dma_start(
                    out=bass.AP(tensor=out.tensor, offset=0, ap=[[NX, NY], [G, 3], [1, NX]]),
                    in_=osb,
                )

    nc.compile()
    return nc


def axes_affine(v):
    """(v0, step) if v is an affine fp32 ramp v0 + i*step (to ~1 ulp), else None."""
    v = np.asarray(v, np.float32)
    n = v.shape[0]
    step = np.float32((float(v[-1]) - float(v[0])) / (n - 1))
    gen = (np.float32(v[0]) + np.arange(n, dtype=np.float32) * step).astype(np.float32)
    span = max(abs(float(v[-1] - v[0])), 1e-30)
    if np.abs(gen - v).max() <= 1e-6 * span:
        return float(v[0]), float(step)
    return None


def make_separable_inputs(X_c, Y_c, ys0, xs0):
    # by[p, ch] = ys0 - Xy[ch*128+p];  bx[p, ch] = xs0 - Xx[ch*128+p]
    xt = X_c.reshape(B, NCH, 128, 2).transpose(0, 2, 1, 3)  # [B,128,NCH,2]
    by = np.float32(ys0) - xt[..., 1]
    bx = np.float32(xs0) - xt[..., 0]
    eyf = np.concatenate([np.ones((B, N, 1), np.float32), Y_c], axis=2)
    ey = np.ascontiguousarray(
        eyf.reshape(B, NCH, 128, 3).transpose(0, 2, 1, 3).reshape(B, 128, 3 * NCH)
    )
    blob = np.concatenate([by, bx, ey], axis=2).astype(np.float32)  # [B,128,40]
    return [{"blob": blob[b]} for b in range(B)]


_program_cache = {}


def _cached_separable(inv2l2, step_y, step_x):
    key = ("sep", inv2l2, step_y, step_x)
    if key not in _program_cache:
        _program_cache[key] = _build_separable(inv2l2, step_y, step_x)
    return _program_cache[key]


def _run_spmd_with_retry(nc, in_maps, tries=3):
    """The axon-tunneled devices sporadically die with
    NRT_EXEC_UNIT_UNRECOVERABLE; a backend reset + retry recovers."""
    import time

    for attempt in range(tries):
        try:
            return run_bass_kernel_spmd(nc, in_maps, list(range(B)))
        except Exception:
            if attempt == tries - 1:
                raise
            try:
                import jax
                import jax.extend as jex

                jax.clear_caches()
                jex.backend.clear_backends()
            except Exception:
                pass
            time.sleep(3.0 * (attempt + 1))


def _run_separable(X_c, Y_c, affy, affx, inv2l2):
    (ys0, step_y), (xs0, step_x) = affy, affx
    nc = _cached_separable(inv2l2, step_y, step_x)
    in_maps = make_separable_inputs(X_c, Y_c, ys0, xs0)
    res = _run_spmd_with_retry(nc, in_maps)
    return np.stack([res.results[b]["out"] for b in range(B)]).astype(np.float32)


def _build_general(inv2l2: float):
    """Fallback for a non-mesh grid: quadratic-form Gram, fused per g-tile.

    Inputs per core: xa[4, 1024] (X augmented), ga[4, G] (grid augmented),
    eyt[128, 3*NCH].  S[n, g] = sum_k xa[k, n] * ga[k, g] = -d2/(2 l^2).
    """
    nc = bacc.Bacc("TRN2", target_bir_lowering=False, debug=False)
    xa = nc.dram_tensor("xa", [4, N], F32, kind="ExternalInput").ap()
    ga = nc.dram_tensor("ga", [4, G], F32, kind="ExternalInput").ap()
    ey = nc.dram_tensor("ey", [128, 3 * NCH], F32, kind="ExternalInput").ap()
    out = nc.dram_tensor("out", [3, G], F32, kind="ExternalOutput").ap()

    GT = 512  # g-tile width
    with tile.TileContext(nc) as tc, ExitStack() as ctx:
        singles = ctx.enter_context(tc.tile_pool(name="singles", bufs=1))
        gpool = ctx.enter_context(tc.tile_pool(name="gpool", bufs=2))
        gram_p = ctx.enter_context(tc.tile_pool(name="gram_p", bufs=2))
        spsum = ctx.enter_context(tc.tile_pool(name="spsum", bufs=2, space="PSUM"))
        fpsum = ctx.enter_context(tc.tile_pool(name="fpsum", bufs=1, space="PSUM"))
        opool = ctx.enter_context(tc.tile_pool(name="opool", bufs=2))

        xa_sb = singles.tile([4, N], F32)
        nc.sync.dma_start(out=xa_sb, in_=xa)
        ey_sb = singles.tile([128, 3 * NCH], F32)
        nc.sync.dma_start(out=ey_sb, in_=ey)

        for gt in range(G // GT):
            ga_sb = gpool.tile([4, GT], F32, tag="ga")
            nc.sync.dma_start(out=ga_sb, in_=ga[:, gt * GT : (gt + 1) * GT])
            # one PSUM tile per 128-g subtile: interleaved matmul accumulation
            # groups must not share a PSUM zero region
            fms = [
                fpsum.tile([128, 3], F32, tag=f"fm{sub}", name=f"fm{sub}_{gt}")
                for sub in range(GT // 128)
            ]
            for ch in range(NCH):
                s = spsum.tile([128, GT], F32, tag="s")
                nc.tensor.matmul(
                    s, lhsT=xa_sb[:, ch * 128 : (ch + 1) * 128], rhs=ga_sb,
                    start=True, stop=True,
                )
                gram = gram_p.tile([128, GT], F32, tag="gram")
                nc.scalar.activation(out=gram, in_=s, func=AF.Exp, scale=1.0)
                for sub in range(GT // 128):
                    nc.tensor.matmul(
                        fms[sub],
                        lhsT=gram[:, sub * 128 : (sub + 1) * 128],
                        rhs=ey_sb[:, 3 * ch : 3 * ch + 3],
                        start=(ch == 0),
                        stop=(ch == NCH - 1),
                    )
            # fms[sub][gp, c] for g = gt*GT + sub*128 + gp
            recip = opool.tile([128, GT // 128], F32, tag="recip")
            osb = opool.tile([128, 3, GT // 128], F32, tag="osb")
            for sub in range(GT // 128):
                nc.vector.reciprocal(
                    out=recip[:, sub : sub + 1], in_=fms[sub][:, 0:1]
                )
                nc.scalar.copy(out=osb[:, 0, sub : sub + 1], in_=fms[sub][:, 0:1])
                for c in (1, 2):
                    nc.vector.tensor_tensor(
                        out=osb[:, c, sub : sub + 1],
                        in0=fms[sub][:, c : c + 1],
                        in1=recip[:, sub : sub + 1],
                        op=ALU.mult,
                    )
            # out[c, g]: g = gt*GT + sub*128 + gp -> per channel dram dims [gp, sub]
            for c in range(3):
                dview = bass.AP(
                    tensor=out.tensor,
                    offset=c * G + gt * GT,
                    ap=[[1, 128], [128, GT // 128]],
                )
                nc.sync.dma_start(out=dview, in_=osb[:, c, :])

    nc.compile()
    return nc


def _run_general(X_c, Y_c, grid, inv2l2):
    nc = _build_general(inv2l2)
    Xs = (X_c * X_c).sum(-1)  # [B, N]
    xa = np.stack(
        [X_c[..., 0], X_c[..., 1], -inv2l2 * Xs, np.ones((B, N), np.float32)], axis=1
    ).astype(np.float32)  # [B, 4, N]
    gs = (grid * grid).sum(-1)  # [G]
    ga = np.stack(
        [
            2.0 * inv2l2 * grid[:, 0],
            2.0 * inv2l2 * grid[:, 1],
            np.ones(G, np.float32),
            -inv2l2 * gs,
        ],
        axis=0,
    ).astype(np.float32)  # [4, G]
    eyf = np.concatenate([np.ones((B, N, 1), np.float32), Y_c], axis=2)
    ey = np.ascontiguousarray(
        eyf.reshape(B, NCH, 128, 3).transpose(0, 2, 1, 3).reshape(B, 128, 3 * NCH)
    )
    in_maps = [{"xa": xa[b], "ga": ga, "ey": ey[b]} for b in range(B)]
    res = _run_spmd_with_retry(nc, in_maps)
    fm = np.stack([res.results[b]["out"] for b in range(B)])  # [B, 3, G]
    return fm.reshape(B, 3, NY, NX).astype(np.float32)


def kernel(X_c, Y_c, grid, log_l_scale):
    X_c = np.asarray(X_c, np.float32)
    Y_c = np.asarray(Y_c, np.float32)
    grid = np.asarray(grid, np.float32)
    l = float(np.exp(np.clip(np.asarray(log_l_scale)[0], -5.0, 5.0)))
    inv2l2 = 1.0 / (2.0 * l * l)

    grid_r = grid.reshape(NY, NX, 2)
    xs = grid_r[0, :, 0].copy()
    ys = grid_r[:, 0, 1].copy()
    separable = np.array_equal(grid_r[..., 0], np.broadcast_to(xs[None, :], (NY, NX))) and (
        np.array_equal(grid_r[..., 1], np.broadcast_to(ys[:, None], (NY, NX)))
    )
    affy, affx = axes_affine(ys), axes_affine(xs)
    if separable and affy is not None and affx is not None:
        return _run_separable(X_c, Y_c, affy, affx, inv2l2)
    return _run_general(X_c, Y_c, grid, inv2l2)


if __name__ == "__main__":
    rng = np.random.default_rng(0)
    ins = {
        "X_c": rng.uniform(0, 10, (B, N, 2)).astype(np.float32),
        "Y_c": rng.normal(size=(B, N, 2)).astype(np.float32),
        "grid": np.stack(
            np.meshgrid(
                np.linspace(0, 10, NY, dtype=np.float32),
                np.linspace(0, 10, NX, dtype=np.float32),
                indexing="ij",
            )[::-1],
            axis=-1,
        ).reshape(-1, 2),
        "log_l_scale": np.zeros(1, np.float32),
    }
    o = kernel(**ins)
    print("out", o.shape, o.dtype)



# revision 30
# speedup vs baseline: 19889.6197x; 1.0552x over previous
"""Trainium2 Bass kernel for EquivDeepSet RBF grid encoder.

Computes, for each batch b:
    Gram[g, n] = exp(-|grid[g] - X_c[b, n]|^2 / (2 l^2))
    FM[g, c]   = sum_n Gram[g, n] * [1, Y_c[b, n, 0], Y_c[b, n, 1]][c]
    out[b]     = [density, feats / density] reshaped to [3, NY, NX]

Sharding: data-parallel over batch across the 8 cores (B == 8).

Fast path exploits that the grid is a tensor-product mesh:
    Gram[(i, j), n] = Ky[i, n] * Kx[j, n]
with Ky/Kx the 1-D RBF factor matrices [128, 1024].  Per core the whole
computation is then: two [128, 1024] elementwise Gaussians, one
[128n, (3c 128j)] weighted moving matrix, and 8 accumulating matmuls
into a single [128i, 3*128] PSUM tile — no [G, N] Gram materialization.

A general (non-mesh grid) fallback computes the Gram tile-by-tile via a
rank-4 quadratic-form matmul and fuses exp + weighted sum per tile.
"""

import numpy as np
from contextlib import ExitStack

import concourse.bacc as bacc
import concourse.bass as bass
import concourse.tile as tile
from concourse import mybir
from concourse.bass_utils import run_bass_kernel_spmd

F32 = mybir.dt.float32
AF = mybir.ActivationFunctionType
ALU = mybir.AluOpType

B, N, NY, NX = 8, 1024, 128, 128
NCH = N // 128  # n-chunks of 128 context points
G = NY * NX


def _fap(base, dims, extra_offset=0):
    """AP with the same tensor/partition dim as `base` but custom free dims.

    dims: list of (step, count) pairs in elements of base's layout.
    """
    return bass.AP(
        tensor=base.tensor,
        offset=base.offset + extra_offset,
        ap=[list(base.ap[0])] + [[s, c] for (s, c) in dims],
    )


def _build_separable(inv2l2: float, step_y: float, step_x: float,
                     sqy_eng=("act", "act", "act", "act", "dve", "act", "dve", "dve"),
                     sqx_eng=("pool", "dve", "pool", "dve", "pool", "dve", "pool", "pool"),
                     exp_pairs=True, loop_k=None, use_fast_recip=True):
    """Per-core program: one batch.

    Inputs: blob[128,40] f32: cols 0:8 = by (ys0 - Xy per chunk),
    8:16 = bx (xs0 - Xx per chunk), 16:40 = expanded-Y fp32.

    The grid axes are affine (ys[i] = ys0 + i*step_y), so a Pool iota ramp
    replaces any axes input: d = step*iota + bias, squared, then
    exp(-inv2l2 * d^2) in bf16.  w[c,j] = ey[c]*kx[j] via bf16
    tensor_scalar; fm[i,(c,j)] accumulates over the 8 n-chunks in PSUM.
    """
    nc = bacc.Bacc("TRN2", target_bir_lowering=False, debug=False)
    BF16 = mybir.dt.bfloat16
    blob = nc.dram_tensor("blob", [128, 40], F32, kind="ExternalInput").ap()
    out = nc.dram_tensor("out", [3, NY, NX], F32, kind="ExternalOutput").ap()

    with tile.TileContext(nc) as tc, ExitStack() as ctx:
        singles = ctx.enter_context(tc.tile_pool(name="singles", bufs=1))
        work = ctx.enter_context(tc.tile_pool(name="work", bufs=3))
        opool = ctx.enter_context(tc.tile_pool(name="opool", bufs=1))
        psum = ctx.enter_context(tc.tile_pool(name="psum", bufs=1, space="PSUM"))

        blob_sb = singles.tile([128, 40], F32)
        nc.sync.dma_start(out=blob_sb, in_=blob)
        ey_sb = blob_sb[:, 16:40]  # [128, 24] fp32

        # Early dummy activations force the (Square|Exp) table load to start
        # before any data dependency, overlapping the blob DMA.
        scr = singles.tile([1, 8], F32)
        nc.vector.memset(scr, 0.0)
        scr2 = singles.tile([1, 8], F32)
        nc.scalar.activation(out=scr2, in_=scr, func=AF.Square, scale=1.0)

        iota = singles.tile([128, 128], F32)
        nc.gpsimd.iota(
            iota, [[1, 128]], channel_multiplier=0,
            allow_small_or_imprecise_dtypes=True,  # 0..127 exact in fp32
        )

        def emit_square(eng_name, d_out, step, bias_ap, tag):
            """d_out = (step*iota + bias)^2 on the chosen engine."""
            if eng_name == "act":
                nc.scalar.activation(
                    out=d_out, in_=iota, func=AF.Square, bias=bias_ap, scale=step
                )
            else:
                eng = nc.gpsimd if eng_name == "pool" else nc.vector
                d = work.tile([128, 128], F32, tag=tag)
                eng.tensor_scalar(
                    out=d, in0=iota, scalar1=step, scalar2=bias_ap,
                    op0=ALU.mult, op1=ALU.add,
                )
                eng.tensor_tensor(out=d_out, in0=d, in1=d, op=ALU.mult)

        loop_ctx = tc.For_i(0, loop_k, 1) if loop_k else None
        if loop_ctx is None:
            fm_d = psum.tile([128, 128], F32, tag="fm_d")
            fm_f = psum.tile([128, 2, 128], F32, tag="fm_f")
            if exp_pairs == "mixed":
                groups = [(0, 1), (2, 3), (4, 5), (6,), (7,)]
            elif exp_pairs:
                groups = [(0, 1), (2, 3), (4, 5), (6, 7)]
            else:
                groups = [(ch,) for ch in range(NCH)]
            for grp in groups:
                per = len(grp)
                dsq = work.tile([128, per, 2, 128], F32, tag=f"dsq{per}")
                kk = work.tile([128, per, 2, 128], BF16, tag=f"kk{per}")
                for half, ch in enumerate(grp):
                    emit_square(sqy_eng[ch], dsq[:, half, 0, :], step_y,
                                blob_sb[:, ch : ch + 1], f"dy{half}")
                    emit_square(sqx_eng[ch], dsq[:, half, 1, :], step_x,
                                blob_sb[:, 8 + ch : 8 + ch + 1], f"dx{half}")
                nc.scalar.activation(
                    out=kk.rearrange("p a b c -> p (a b c)"),
                    in_=dsq.rearrange("p a b c -> p (a b c)"),
                    func=AF.Exp,
                    scale=-inv2l2,
                )
                for half, ch in enumerate(grp):
                    # density channel: ey[:, 3ch] == 1, so rhs is Kx itself
                    nc.tensor.matmul(
                        fm_d,
                        lhsT=kk[:, half, 0, :],
                        rhs=kk[:, half, 1, :],
                        start=(ch == 0),
                        stop=(ch == NCH - 1),
                    )
                    w = work.tile([128, 2, 128], BF16, tag="w")
                    for c in (1, 2):
                        nc.vector.tensor_scalar(
                            out=w[:, c - 1, :],
                            in0=kk[:, half, 1, :],
                            scalar1=ey_sb[:, 3 * ch + c : 3 * ch + c + 1],
                            scalar2=None,
                            op0=ALU.mult,
                        )
                    nc.tensor.matmul(
                        fm_f.rearrange("p c j -> p (c j)"),
                        lhsT=kk[:, half, 0, :],
                        rhs=w.rearrange("p c j -> p (c j)"),
                        start=(ch == 0),
                        stop=(ch == NCH - 1),
                    )

            # normalize: out0 = density, out1/2 = feats * (1/density)
            osb = opool.tile([128, 3, 128], F32, tag="osb")
            nc.scalar.copy(out=osb[:, 0, :], in_=fm_d)
            recip = opool.tile([128, 128], F32, tag="recip")
            if use_fast_recip:
                nc.vector.reciprocal_approx_fast(out=recip, in_=fm_d)
            else:
                nc.vector.reciprocal(out=recip, in_=fm_d)
            nc.vector.scalar_tensor_tensor(
                out=osb[:, 1:3, :],
                in0=fm_f,
                scalar=1.0,
                in1=_fap(recip, [(0, 2), (1, 128)]),
                op0=ALU.mult,
                op1=ALU.mult,
            )
            nc.sync.dma_start(
                out=bass.AP(tensor=out.tensor, offset=0, ap=[[NX, NY], [G, 3], [1, NX]]),
                in_=osb,
            )
        else:
            with loop_ctx:
                fm_d = psum.tile([128, 128], F32, tag="fm_d")
                fm_f = psum.tile([128, 2, 128], F32, tag="fm_f")
                if exp_pairs == "mixed":
                    groups = [(0, 1), (2, 3), (4, 5), (6,), (7,)]
                elif exp_pairs:
                    groups = [(0, 1), (2, 3), (4, 5), (6, 7)]
                else:
                    groups = [(ch,) for ch in range(NCH)]
                for grp in groups:
                    per = len(grp)
                    dsq = work.tile([128, per, 2, 128], F32, tag=f"dsq{per}")
                    kk = work.tile([128, per, 2, 128], BF16, tag=f"kk{per}")
                    for half, ch in enumerate(grp):
                        emit_square(sqy_eng[ch], dsq[:, half, 0, :], step_y,
                                    blob_sb[:, ch : ch + 1], f"dy{half}")
                        emit_square(sqx_eng[ch], dsq[:, half, 1, :], step_x,
                                    blob_sb[:, 8 + ch : 8 + ch + 1], f"dx{half}")
                    nc.scalar.activation(
                        out=kk.rearrange("p a b c -> p (a b c)"),
                        in_=dsq.rearrange("p a b c -> p (a b c)"),
                        func=AF.Exp,
                        scale=-inv2l2,
                    )
                    for half, ch in enumerate(grp):
                        # density channel: ey[:, 3ch] == 1, so rhs is Kx itself
                        nc.tensor.matmul(
                            fm_d,
                            lhsT=kk[:, half, 0, :],
                            rhs=kk[:, half, 1, :],
                            start=(ch == 0),
                            stop=(ch == NCH - 1),
                        )
                        w = work.tile([128, 2, 128], BF16, tag="w")
                        for c in (1, 2):
                            nc.vector.tensor_scalar(
                                out=w[:, c - 1, :],
                                in0=kk[:, half, 1, :],
                                scalar1=ey_sb[:, 3 * ch + c : 3 * ch + c + 1],
                                scalar2=None,
                                op0=ALU.mult,
                            )
                        nc.tensor.matmul(
                            fm_f.rearrange("p c j -> p (c j)"),
                            lhsT=kk[:, half, 0, :],
                            rhs=w.rearrange("p c j -> p (c j)"),
                            start=(ch == 0),
                            stop=(ch == NCH - 1),
                        )

                # normalize: out0 = density, out1/2 = feats * (1/density)
                osb = opool.tile([128, 3, 128], F32, tag="osb")
                nc.scalar.copy(out=osb[:, 0, :], in_=fm_d)
                recip = opool.tile([128, 128], F32, tag="recip")
                if use_fast_recip:
                    nc.vector.reciprocal_approx_fast(out=recip, in_=fm_d)
                else:
                    nc.vector.reciprocal(out=recip, in_=fm_d)
                nc.vector.scalar_tensor_tensor(
                    out=osb[:, 1:3, :],
                    in0=fm_f,
                    scalar=1.0,
                    in1=_fap(recip, [(0, 2), (1, 128)]),
                    op0=ALU.mult,
                    op1=ALU.mult,
                )
                nc.sync.dma_start(
                    out=bass.AP(tensor=out.tensor, offset=0, ap=[[NX, NY], [G, 3], [1, NX]]),
                    in_=osb,
                )

    nc.compile()
    return nc


def axes_affine(v):
    """(v0, step) if v is an affine fp32 ramp v0 + i*step (to ~1 ulp), else None."""
    v = np.asarray(v, np.float32)
    n = v.shape[0]
    step = np.float32((float(v[-1]) - float(v[0])) / (n - 1))
    gen = (np.float32(v[0]) + np.arange(n, dtype=np.float32) * step).astype(np.float32)
    span = max(abs(float(v[-1] - v[0])), 1e-30)
    if np.abs(gen - v).max() <= 1e-6 * span:
        return float(v[0]), float(step)
    return None


def make_separable_inputs(X_c, Y_c, ys0, xs0):
    # by[p, ch] = ys0 - Xy[ch*128+p];  bx[p, ch] = xs0 - Xx[ch*128+p]
    xt = X_c.reshape(B, NCH, 128, 2).transpose(0, 2, 1, 3)  # [B,128,NCH,2]
    by = np.float32(ys0) - xt[..., 1]
    bx = np.float32(xs0) - xt[..., 0]
    eyf = np.concatenate([np.ones((B, N, 1), np.float32), Y_c], axis=2)
    ey = np.ascontiguousarray(
        eyf.reshape(B, NCH, 128, 3).transpose(0, 2, 1, 3).reshape(B, 128, 3 * NCH)
    )
    blob = np.concatenate([by, bx, ey], axis=2).astype(np.float32)  # [B,128,40]
    return [{"blob": blob[b]} for b in range(B)]


_program_cache = {}


def _cached_separable(inv2l2, step_y, step_x):
    key = ("sep", inv2l2, step_y, step_x)
    if key not in _program_cache:
        _program_cache[key] = _build_separable(inv2l2, step_y, step_x)
    return _program_cache[key]


_runner_cache = {}


def _make_runner(nc, n_cores):
    """Persistent jitted shard_map executor for `nc` (same lowering as
    bass2jax.run_bass_via_pjrt, but the jit callable is built once, so
    repeat kernel() calls skip re-tracing)."""
    import jax
    from jax.sharding import Mesh, PartitionSpec
    from jax.experimental.shard_map import shard_map
    from concourse import bass2jax

    bass2jax.install_neuronx_cc_hook()
    partition_name = nc.partition_id_tensor.name if nc.partition_id_tensor else None
    in_names, out_names, out_avals, zero_shapes = [], [], [], []
    for alloc in nc.m.functions[0].allocations:
        if not isinstance(alloc, mybir.MemoryLocationSet):
            continue
        name = alloc.memorylocations[0].name
        if alloc.kind == "ExternalInput":
            if name != partition_name:
                in_names.append(name)
        elif alloc.kind == "ExternalOutput":
            out_names.append(name)
            shape = tuple(alloc.tensor_shape)
            dtype = mybir.dt.np(alloc.dtype)
            out_avals.append(jax.core.ShapedArray(shape, dtype))
            zero_shapes.append((shape, dtype))
    n_params, n_outs = len(in_names), len(out_avals)
    all_in = list(in_names) + list(out_names) + (
        [partition_name] if partition_name else []
    )

    def _body(*args):
        operands = list(args)
        if partition_name is not None:
            operands.append(bass2jax.partition_id_tensor())
        return tuple(
            bass2jax._bass_exec_p.bind(
                *operands,
                out_avals=tuple(out_avals),
                in_names=tuple(all_in),
                out_names=tuple(out_names),
                lowering_input_output_aliases=(),
                sim_require_finite=True,
                sim_require_nnan=True,
                nc=nc,
            )
        )

    devices = jax.devices()[:n_cores]
    mesh = Mesh(np.asarray(devices), ("core",))
    specs = (PartitionSpec("core"),)
    sharded = jax.jit(
        shard_map(_body, mesh=mesh, in_specs=specs * (n_params + n_outs),
                  out_specs=specs * n_outs, check_rep=False),
        donate_argnums=tuple(range(n_params, n_params + n_outs)),
        keep_unused=True,
    )

    def run(in_maps):
        per_core = [[np.asarray(m[name]) for name in in_names] for m in in_maps]
        concat_in = [
            np.concatenate([per_core[c][i] for c in range(n_cores)], axis=0)
            for i in range(n_params)
        ]
        concat_zeros = [
            np.zeros((n_cores * s[0], *s[1:]), dt) for (s, dt) in zero_shapes
        ]
        outs = [np.asarray(a) for a in sharded(*concat_in, *concat_zeros)]
        return [
            {
                name: outs[i].reshape(n_cores, *out_avals[i].shape)[c]
                for i, name in enumerate(out_names)
            }
            for c in range(n_cores)
        ]

    return run


def _run_program(nc, in_maps):
    """Execute via a cached jitted runner; fall back to run_bass_kernel_spmd."""
    key = id(nc)
    try:
        if key not in _runner_cache:
            _runner_cache[key] = _make_runner(nc, len(in_maps))
        return _runner_cache[key](in_maps)
    except Exception:
        _runner_cache.pop(key, None)
        return run_bass_kernel_spmd(nc, in_maps, list(range(len(in_maps)))).results


def _run_spmd_with_retry(nc, in_maps, tries=3):
    """The axon-tunneled devices sporadically die with
    NRT_EXEC_UNIT_UNRECOVERABLE; a backend reset + retry recovers."""
    import time

    for attempt in range(tries):
        try:
            return _run_program(nc, in_maps)
        except Exception:
            if attempt == tries - 1:
                raise
            try:
                import jax
                import jax.extend as jex

                jax.clear_caches()
                jex.backend.clear_backends()
            except Exception:
                pass
            time.sleep(3.0 * (attempt + 1))


def _run_separable(X_c, Y_c, affy, affx, inv2l2):
    (ys0, step_y), (xs0, step_x) = affy, affx
    nc = _cached_separable(inv2l2, step_y, step_x)
    in_maps = make_separable_inputs(X_c, Y_c, ys0, xs0)
    res = _run_spmd_with_retry(nc, in_maps)
    return np.stack([res[b]["out"] for b in range(B)]).astype(np.float32)


def _build_general(inv2l2: float):
    """Fallback for a non-mesh grid: quadratic-form Gram, fused per g-tile.

    Inputs per core: xa[4, 1024] (X augmented), ga[4, G] (grid augmented),
    eyt[128, 3*NCH].  S[n, g] = sum_k xa[k, n] * ga[k, g] = -d2/(2 l^2).
    """
    nc = bacc.Bacc("TRN2", target_bir_lowering=False, debug=False)
    xa = nc.dram_tensor("xa", [4, N], F32, kind="ExternalInput").ap()
    ga = nc.dram_tensor("ga", [4, G], F32, kind="ExternalInput").ap()
    ey = nc.dram_tensor("ey", [128, 3 * NCH], F32, kind="ExternalInput").ap()
    out = nc.dram_tensor("out", [3, G], F32, kind="ExternalOutput").ap()

    GT = 512  # g-tile width
    with tile.TileContext(nc) as tc, ExitStack() as ctx:
        singles = ctx.enter_context(tc.tile_pool(name="singles", bufs=1))
        gpool = ctx.enter_context(tc.tile_pool(name="gpool", bufs=2))
        gram_p = ctx.enter_context(tc.tile_pool(name="gram_p", bufs=2))
        spsum = ctx.enter_context(tc.tile_pool(name="spsum", bufs=2, space="PSUM"))
        fpsum = ctx.enter_context(tc.tile_pool(name="fpsum", bufs=1, space="PSUM"))
        opool = ctx.enter_context(tc.tile_pool(name="opool", bufs=2))

        xa_sb = singles.tile([4, N], F32)
        nc.sync.dma_start(out=xa_sb, in_=xa)
        ey_sb = singles.tile([128, 3 * NCH], F32)
        nc.sync.dma_start(out=ey_sb, in_=ey)

        for gt in range(G // GT):
            ga_sb = gpool.tile([4, GT], F32, tag="ga")
            nc.sync.dma_start(out=ga_sb, in_=ga[:, gt * GT : (gt + 1) * GT])
            # one PSUM tile per 128-g subtile: interleaved matmul accumulation
            # groups must not share a PSUM zero region
            fms = [
                fpsum.tile([128, 3], F32, tag=f"fm{sub}", name=f"fm{sub}_{gt}")
                for sub in range(GT // 128)
            ]
            for ch in range(NCH):
                s = spsum.tile([128, GT], F32, tag="s")
                nc.tensor.matmul(
                    s, lhsT=xa_sb[:, ch * 128 : (ch + 1) * 128], rhs=ga_sb,
                    start=True, stop=True,
                )
                gram = gram_p.tile([128, GT], F32, tag="gram")
                nc.scalar.activation(out=gram, in_=s, func=AF.Exp, scale=1.0)
                for sub in range(GT // 128):
                    nc.tensor.matmul(
                        fms[sub],
                        lhsT=gram[:, sub * 128 : (sub + 1) * 128],
                        rhs=ey_sb[:, 3 * ch : 3 * ch + 3],
                        start=(ch == 0),
                        stop=(ch == NCH - 1),
                    )
            # fms[sub][gp, c] for g = gt*GT + sub*128 + gp
            recip = opool.tile([128, GT // 128], F32, tag="recip")
            osb = opool.tile([128, 3, GT // 128], F32, tag="osb")
            for sub in range(GT // 128):
                nc.vector.reciprocal(
                    out=recip[:, sub : sub + 1], in_=fms[sub][:, 0:1]
                )
                nc.scalar.copy(out=osb[:, 0, sub : sub + 1], in_=fms[sub][:, 0:1])
                for c in (1, 2):
                    nc.vector.tensor_tensor(
                        out=osb[:, c, sub : sub + 1],
                        in0=fms[sub][:, c : c + 1],
                        in1=recip[:, sub : sub + 1],
                        op=ALU.mult,
                    )
            # out[c, g]: g = gt*GT + sub*128 + gp -> per channel dram dims [gp, sub]
            for c in range(3):
                dview = bass.AP(
                    tensor=out.tensor,
                    offset=c * G + gt * GT,
                    ap=[[1, 128], [128, GT // 128]],
                )
                nc.sync.dma_start(out=dview, in_=osb[:, c, :])

    nc.compile()
    return nc


def _run_general(X_c, Y_c, grid, inv2l2):
    nc = _build_general(inv2l2)
    Xs = (X_c * X_c).sum(-1)  # [B, N]
    xa = np.stack(
        [X_c[..., 0], X_c[..., 1], -inv2l2 * Xs, np.ones((B, N), np.float32)], axis=1
    ).astype(np.float32)  # [B, 4, N]
    gs = (grid * grid).sum(-1)  # [G]
    ga = np.stack(
        [
            2.0 * inv2l2 * grid[:, 0],
            2.0 * inv2l2 * grid[:, 1],
            np.ones(G, np.float32),
            -inv2l2 * gs,
        ],
        axis=0,
    ).astype(np.float32)  # [4, G]
    eyf = np.concatenate([np.ones((B, N, 1), np.float32), Y_c], axis=2)
    ey = np.ascontiguousarray(
        eyf.reshape(B, NCH, 128, 3).transpose(0, 2, 1, 3).reshape(B, 128, 3 * NCH)
    )
    in_maps = [{"xa": xa[b], "ga": ga, "ey": ey[b]} for b in range(B)]
    res = _run_spmd_with_retry(nc, in_maps)
    fm = np.stack([res[b]["out"] for b in range(B)])  # [B, 3, G]
    return fm.reshape(B, 3, NY, NX).astype(np.float32)


def kernel(X_c, Y_c, grid, log_l_scale):
    X_c = np.asarray(X_c, np.float32)
    Y_c = np.asarray(Y_c, np.float32)
    grid = np.asarray(grid, np.float32)
    l = float(np.exp(np.clip(np.asarray(log_l_scale)[0], -5.0, 5.0)))
    inv2l2 = 1.0 / (2.0 * l * l)

    grid_r = grid.reshape(NY, NX, 2)
    xs = grid_r[0, :, 0].copy()
    ys = grid_r[:, 0, 1].copy()
    separable = np.array_equal(grid_r[..., 0], np.broadcast_to(xs[None, :], (NY, NX))) and (
        np.array_equal(grid_r[..., 1], np.broadcast_to(ys[:, None], (NY, NX)))
    )
    affy, affx = axes_affine(ys), axes_affine(xs)
    if separable and affy is not None and affx is not None:
        return _run_separable(X_c, Y_c, affy, affx, inv2l2)
    return _run_general(X_c, Y_c, grid, inv2l2)


if __name__ == "__main__":
    rng = np.random.default_rng(0)
    ins = {
        "X_c": rng.uniform(0, 10, (B, N, 2)).astype(np.float32),
        "Y_c": rng.normal(size=(B, N, 2)).astype(np.float32),
        "grid": np.stack(
            np.meshgrid(
                np.linspace(0, 10, NY, dtype=np.float32),
                np.linspace(0, 10, NX, dtype=np.float32),
                indexing="ij",
            )[::-1],
            axis=-1,
        ).reshape(-1, 2),
        "log_l_scale": np.zeros(1, np.float32),
    }
    o = kernel(**ins)
    print("out", o.shape, o.dtype)

